# revision 8
# baseline (speedup 1.0000x reference)
"""AnchorAttention distributed Bass kernel for 8 TRN2 NeuronCores.

Problem: x:(2, 8192, 1024) f32; first A=1024 tokens per batch are anchors.
  aqkv = anchors @ Wqkv -> per-head aq, ak, av     (H=16 heads, hd=64)
  qq   = queries @ Wq   -> per-head q
  out  = softmax((concat(aq,qq) @ ak^T)/8) @ av, reshaped, @ Wproj

Sharding: sequence-parallel. Core c in 0..7 owns 2048 token rows:
  batch b = c // 4, rows [2048*(c%4), 2048*(c%4)+2048) of that batch.
Each query row attends only to its batch's anchors, so no collectives are
needed; anchor K/V is recomputed on each core (small). Host pre-transposes
and pre-casts inputs to bf16; rel-err tolerance is ~2e-2 and bf16 compute
lands ~3e-3.

Device kernel layout choices (everything transposed so matmuls chain):
  xT (D, 2048), aT (D, 1024) in DRAM.
  KT[dk, a]   = (anchors @ Wk)^T        via out^T = WkT-tile-stationary mm
  V[a, dv]    = anchors @ Wv            natural
  QT[dq, q]   = (rows @ Wq_eff)^T
  scores^T[a_tile, q] = KT-tile^T @ QT  (K=hd=64; even/odd heads auto
                       row-tile to array rows 0-63 / 64-127 and write
                       different PSUM banks)
  ST = exp(scores * 1/8)  on ScalarE, PSUM->SBUF bf16   (max-free softmax:
                       scores are O(+-8) so exp is safe in f32/bf16)
  attn-out^T[hd, q] accumulated over anchor tiles: lhsT = V tile, the head
                       pair col-packed via tile_position (0,0)/(0,64)
  denom[q] = ones^T @ ST (M=1 matmuls, col-packed to psum partitions 0/64);
  both PSUM accumulators are evacuated immediately (copy / reciprocal) so
  the next pair never stalls on the normalize chain; recips are partition-
  broadcast via a DRAM bounce (stride-0 partition DMA is DRAM-source only)
  and applied as one tensor_tensor multiply into AO.
  final out[q, do]: lhsT = AO tile, rhs = Wproj, f32 DMA out.

  The whole kernel is software-pipelined in 4 q-rounds of 512 columns:
  round qb interleaves attention (ACT-bound: 64 exps of [128,1024]) with
  the NEXT round's Q-projection matmuls and the PREVIOUS round's output-
  projection matmuls, so the PE work of those phases hides under the
  ScalarE exp stream. KT projection rides inside round 0; the V projection
  and first Q-projection block form the prologue.
"""

import sys

if "/opt/trn_rl_repo" not in sys.path:
    sys.path.insert(0, "/opt/trn_rl_repo")

import numpy as np
import ml_dtypes

from concourse import bacc, mybir, tile
from concourse.bass_utils import run_bass_kernel_spmd

# ---------------------------------------------------------------- constants
B, S, D = 2, 8192, 1024
H, HD, A = 16, 64, 1024
NQ = 2048          # token rows per core
NCORES = 8
DT = D // 128      # 8 x 128-row tiles of the model dim
AT = A // 128      # 8 anchor tiles
QB = 512           # q block inside attention
NPAIR = H // 2     # head pairs (adjacent heads share a 128-partition tile)
SCALE = 1.0 / 8.0  # 1/sqrt(hd)

F32 = mybir.dt.float32
BF16 = mybir.dt.bfloat16

_cached_nc = None


def build_kernel(repeat=1):
    nc = bacc.Bacc("TRN2", target_bir_lowering=False, debug=False,
                   num_devices=NCORES)

    xT = nc.declare_dram_parameter("xT", [D, NQ], BF16, isOutput=False)
    aT = nc.declare_dram_parameter("aT", [D, A], BF16, isOutput=False)
    wq0 = nc.declare_dram_parameter("wq0", [D, D], BF16, isOutput=False)
    wq1 = nc.declare_dram_parameter("wq1", [D, D], BF16, isOutput=False)
    wkv = nc.declare_dram_parameter("wkv", [D, 2 * D], BF16, isOutput=False)
    wpr = nc.declare_dram_parameter("wpr", [D, D], BF16, isOutput=False)
    # Q bias, column 0 for q<1024 rows, column 1 for the rest (exact, f32)
    bq2 = nc.declare_dram_parameter("bq2", [D, 2], F32, isOutput=False)
    out = nc.declare_dram_parameter("out", [NQ, D], F32, isOutput=True)

    NQB = NQ // QB  # 4 q rounds

    with tile.TileContext(nc) as tc:
        for _rep in range(repeat):
          with (
            tc.tile_pool(name="attn", bufs=1) as p_attn,      # KT, V, QT
            tc.tile_pool(name="ao", bufs=1) as p_ao,          # AO blocks, WP
            tc.tile_pool(name="stage", bufs=1) as p_stage,    # aT
            tc.tile_pool(name="xq", bufs=2) as p_xq,          # x panel / round
            tc.tile_pool(name="wt", bufs=2) as p_w,           # weight panels
            tc.tile_pool(name="st", bufs=4) as p_st,
            tc.tile_pool(name="small", bufs=1) as p_small,
            tc.tile_pool(name="pvs", bufs=4) as p_pvs,
            tc.tile_pool(name="rcb", bufs=3) as p_rcb,
            tc.tile_pool(name="scr", bufs=6, space="DRAM") as p_scr,
            tc.tile_pool(name="outsb", bufs=2) as p_out,
            tc.tile_pool(name="psps", bufs=2, space="PSUM") as ps_s,
            tc.tile_pool(name="psacc", bufs=2, space="PSUM") as ps_acc,
            tc.tile_pool(name="psv", bufs=1, space="PSUM") as ps_v,
          ):
            KT = p_attn.tile([128, DT, A], BF16, tag="KT")
            # V with a ones column appended per head (65 cols/head): the AV
            # matmul (M=65) then yields the softmax denominator as PSUM row
            # 64 for free, replacing the former M=1 ones-matmuls which cost
            # full N-cycle streams on the PE.
            V = p_attn.tile([128, AT, H, 65], BF16, tag="V")
            nc.vector.memset(V[:, :, :, 64:65], 1.0)
            QT = p_attn.tile([128, DT, NQ], BF16, tag="QT")
            AOq = []
            for i in range(NQB):
                ao_i = p_ao.tile([128, DT, QB], BF16, tag=f"AO{i}",
                                 name=f"AO{i}")
                AOq.append(ao_i)
            WP = p_ao.tile([128, DT, D], BF16, tag="WP")
            bqs = p_small.tile([128, DT, 2], F32, tag="bqs")
            nc.sync.dma_start(
                out=bqs[:], in_=bq2[:].rearrange("(k p) c -> p k c", p=128))
            nc.gpsimd.dma_start(
                out=WP[:], in_=wpr[:].rearrange("(k p) c -> p k c", p=128))
            aTs = p_stage.tile([128, DT, A], BF16, tag="aT")
            nc.scalar.dma_start(
                out=aTs[:], in_=aT[:].rearrange("(k p) a -> p k a", p=128))

            # ---------------- emission helpers ---------------------------
            def emit_V_panel(vh):
                wv = p_w.tile([128, DT, 512], BF16, tag="wv", name=f"wv{vh}")
                nc.gpsimd.dma_start(
                    out=wv[:],
                    in_=wkv[:, D + vh * 512:D + (vh + 1) * 512].rearrange(
                        "(k p) c -> p k c", p=128))
                return wv

            def emit_V_at(vh, wv, at):
                acc = ps_acc.tile([128, 512], F32, tag="acc")
                for dn in range(DT):
                    nc.tensor.matmul(
                        acc[:],
                        lhsT=aTs[:, dn, at * 128:(at + 1) * 128],
                        rhs=wv[:, dn, :],
                        start=(dn == 0), stop=(dn == DT - 1))
                nc.vector.tensor_copy(
                    V[:, at, vh * 8:(vh + 1) * 8, 0:64],
                    acc[:].rearrange("p (h c) -> p h c", h=8))

            def emit_KT(dk):
                wk = p_w.tile([128, DT, 128], BF16, tag="wk")
                nc.gpsimd.dma_start(
                    out=wk[:],
                    in_=wkv[:, dk * 128:(dk + 1) * 128].rearrange(
                        "(k p) c -> p k c", p=128))
                for ah in range(2):
                    acc = ps_acc.tile([128, 512], F32, tag="acc")
                    for dn in range(DT):
                        nc.tensor.matmul(
                            acc[:],
                            lhsT=wk[:, dn, :],
                            rhs=aTs[:, dn, ah * 512:(ah + 1) * 512],
                            start=(dn == 0), stop=(dn == DT - 1))
                    nc.vector.tensor_copy(
                        KT[:, dk, ah * 512:(ah + 1) * 512], acc[:])

            def emit_xq(qs):
                xq = p_xq.tile([128, DT, QB], BF16, tag="xq")
                nc.scalar.dma_start(
                    out=xq[:],
                    in_=xT[:, qs * QB:(qs + 1) * QB].rearrange(
                        "(k p) q -> p k q", p=128))
                return xq

            def emit_C_dq(qs, dq, xq):
                half = 0 if qs < 2 else 1
                wsrc = wq0 if half == 0 else wq1
                wq_t = p_w.tile([128, DT, 128], BF16, tag="wq")
                nc.gpsimd.dma_start(
                    out=wq_t[:],
                    in_=wsrc[:, dq * 128:(dq + 1) * 128].rearrange(
                        "(k p) c -> p k c", p=128))
                acc = ps_acc.tile([128, 512], F32, tag="acc")
                for dn in range(DT):
                    nc.tensor.matmul(
                        acc[:],
                        lhsT=wq_t[:, dn, :],
                        rhs=xq[:, dn, :],
                        start=(dn == 0), stop=(dn == DT - 1))
                nc.vector.tensor_scalar_add(
                    QT[:, dq, qs * QB:(qs + 1) * QB], acc[:],
                    bqs[:, dq, half:half + 1])

            def emit_scores_exp(qb, g, at):
                ps = ps_s.tile([128, 2 * QB], F32, tag="ps")
                for hl in range(2):
                    nc.tensor.matmul(
                        ps[:, hl * QB:(hl + 1) * QB],
                        lhsT=KT[hl * 64:(hl + 1) * 64, g,
                                at * 128:(at + 1) * 128],
                        rhs=QT[hl * 64:(hl + 1) * 64, g,
                               qb * QB:(qb + 1) * QB],
                        start=True, stop=True)
                st = p_st.tile([128, 2 * QB], BF16, tag="st")
                nc.scalar.activation(
                    st[:], ps[:], mybir.ActivationFunctionType.Exp,
                    scale=SCALE)
                return st

            def emit_av(g, at, st, pv0, pv1):
                # M=65 per head: rows 0-63 attn-out, row 64 denominator
                # (ones column of V). Cost is N cycles regardless of M.
                for hl, pv in ((0, pv0), (1, pv1)):
                    h = 2 * g + hl
                    nc.tensor.matmul(
                        pv[:, :],
                        lhsT=V[:, at, h, :],
                        rhs=st[:, hl * QB:(hl + 1) * QB],
                        start=(at == 0), stop=(at == AT - 1))

            def emit_normalize(qb, g, pv0, pv1):
                # evacuate PSUM promptly; normalize downstream on SBUF
                pvs = p_pvs.tile([128, QB], F32, tag="pvs")
                nc.vector.tensor_copy(pvs[0:64, :], pv0[0:64, :])
                nc.vector.tensor_copy(pvs[64:128, :], pv1[0:64, :])
                # DVE partition offsets must be 0/32/64/96-aligned: denom
                # recips land on rows 0 and 64.
                rc = p_pvs.tile([65, QB], F32, tag="rcp")
                nc.vector.reciprocal(rc[0:1, :], pv0[64:65, :])
                nc.vector.reciprocal(rc[64:65, :], pv1[64:65, :])
                scr = p_scr.tile([2, QB], F32, tag="scr")
                nc.sync.dma_start(out=scr[0:1, :], in_=rc[0:1, :])
                nc.sync.dma_start(out=scr[1:2, :], in_=rc[64:65, :])
                rb = p_rcb.tile([128, QB], F32, tag="rb")
                for hl in range(2):
                    nc.sync.dma_start(
                        out=rb[hl * 64:(hl + 1) * 64, :],
                        in_=scr[hl:hl + 1, :].to_broadcast((64, QB)))
                nc.vector.tensor_tensor(
                    out=AOq[qb][:, g, :], in0=pvs[:], in1=rb[:],
                    op=mybir.AluOpType.mult)

            def emit_attention(qb, g, at_hook=None):
                pv0 = ps_v.tile([65, QB], F32, tag="pv0", name="pv0")
                pv1 = ps_v.tile([65, QB], F32, tag="pv1", name="pv1")
                for at in range(AT):
                    if at_hook is not None:
                        at_hook(at)
                    ps = ps_s.tile([128, 2 * QB], F32, tag="ps")
                    # scores^T: even head on PE rows 0-63, odd on 64-127
                    # (auto row-tiling), different PSUM banks.
                    for hl in range(2):
                        nc.tensor.matmul(
                            ps[:, hl * QB:(hl + 1) * QB],
                            lhsT=KT[hl * 64:(hl + 1) * 64, g,
                                    at * 128:(at + 1) * 128],
                            rhs=QT[hl * 64:(hl + 1) * 64, g,
                                   qb * QB:(qb + 1) * QB],
                            start=True, stop=True)
                    st = p_st.tile([128, 2 * QB], BF16, tag="st")
                    nc.scalar.activation(
                        st[:], ps[:], mybir.ActivationFunctionType.Exp,
                        scale=SCALE)
                    emit_av(g, at, st, pv0, pv1)
                emit_normalize(qb, g, pv0, pv1)

            def emit_E(qbi, qi):
                # output rows qt = qbi*4 + qi (128 rows)
                ot = p_out.tile([128, D], F32, tag="ot")
                for dh in range(2):
                    acc = ps_acc.tile([128, 512], F32, tag="acc")
                    for dv in range(DT):
                        nc.tensor.matmul(
                            acc[:],
                            lhsT=AOq[qbi][:, dv, qi * 128:(qi + 1) * 128],
                            rhs=WP[:, dv, dh * 512:(dh + 1) * 512],
                            start=(dv == 0), stop=(dv == DT - 1))
                    nc.vector.tensor_copy(
                        ot[:, dh * 512:(dh + 1) * 512], acc[:])
                qt = qbi * (QB // 128) + qi
                nc.sync.dma_start(
                    out=out[qt * 128:(qt + 1) * 128, :], in_=ot[:])

            # ---------------- software-pipelined schedule ----------------
            # JIT Q-projection: round qb emits C(qb, dq=g+1) one iteration
            # ahead of its own use; only C(qb, 0) crosses the round edge.
            # This keeps the prologue (ACT-idle) to V + KT(0) + C(0,0).
            wvs = [emit_V_panel(0), emit_V_panel(1)]
            xqs = [emit_xq(0)]
            emit_KT(0)
            emit_C_dq(0, 0, xqs[0])
            # skewed (qb=0, g=0): scores+exps interleave with the V
            # projection so ScalarE works during the otherwise-idle
            # prologue. g=0 reads only the vh=0 half of V, so its attn@V
            # follows each V block with a one-tile lag (hides the V
            # PSUM->SBUF copy); the vh=1 V panel is emitted after and is
            # first needed at g=4.
            pva = ps_v.tile([65, QB], F32, tag="pv0", name="pva")
            pvb = ps_v.tile([65, QB], F32, tag="pv1", name="pvb")
            sts0 = []
            for at in range(AT):
                sts0.append(emit_scores_exp(0, 0, at))
                emit_V_at(0, wvs[0], at)
                if at > 0:
                    emit_av(0, at - 1, sts0[at - 1], pva, pvb)
            emit_av(0, AT - 1, sts0[AT - 1], pva, pvb)
            for at in range(AT):
                emit_V_at(1, wvs[1], at)
            emit_normalize(0, 0, pva, pvb)
            emit_KT(1)
            emit_C_dq(0, 1, xqs[0])
            for qb in range(NQB):
                if qb + 1 < NQB:
                    xqs.append(emit_xq(qb + 1))
                for g in range(1 if qb == 0 else 0, NPAIR):
                    if qb == 0 and g + 1 < NPAIR:
                        emit_KT(g + 1)
                    emit_attention(qb, g)
                    if g + 1 < NPAIR:
                        emit_C_dq(qb, g + 1, xqs[qb])
                    elif qb + 1 < NQB:
                        emit_C_dq(qb + 1, 0, xqs[qb + 1])
                    if qb >= 1 and g < QB // 128:
                        emit_E(qb - 1, g)
            for qi in range(QB // 128):
                emit_E(NQB - 1, qi)

    nc.compile()
    return nc


def shard_inputs(x, Wqkv, Wq, Wproj, bqkv, bq):
    """Build per-core in_maps (bf16, pre-transposed; biases f32)."""
    bf = ml_dtypes.bfloat16
    xtb = np.ascontiguousarray(np.transpose(x, (0, 2, 1))).astype(bf)  # (B,D,S)
    wq_anchor = np.ascontiguousarray(Wqkv[:, :D]).astype(bf)
    wq_plain = np.ascontiguousarray(Wq).astype(bf)
    wkv_b = np.ascontiguousarray(Wqkv[:, D:3 * D]).astype(bf)
    wpr_b = np.ascontiguousarray(Wproj).astype(bf)
    bq_anchor = np.asarray(bqkv[:D], np.float32)
    bq_plain = np.asarray(bq, np.float32)
    bq2_anchor = np.ascontiguousarray(
        np.stack([bq_anchor, bq_plain], axis=1))
    bq2_plain = np.ascontiguousarray(
        np.stack([bq_plain, bq_plain], axis=1))
    in_maps = []
    for c in range(NCORES):
        b, blk = c // 4, c % 4
        q0 = blk * NQ
        in_maps.append({
            "xT": np.ascontiguousarray(xtb[b, :, q0:q0 + NQ]),
            "aT": np.ascontiguousarray(xtb[b, :, :A]),
            "wq0": wq_anchor if blk == 0 else wq_plain,
            "wq1": wq_plain,
            "wkv": wkv_b,
            "wpr": wpr_b,
            "bq2": bq2_anchor if blk == 0 else bq2_plain,
        })
    return in_maps


def kernel(x, Wqkv, bqkv, Wq, bq, Wproj, bproj, num_anchor_tokens):
    global _cached_nc
    x = np.asarray(x, dtype=np.float32)
    Wqkv = np.asarray(Wqkv, dtype=np.float32)
    Wq = np.asarray(Wq, dtype=np.float32)
    Wproj = np.asarray(Wproj, dtype=np.float32)
    assert int(num_anchor_tokens) == A and x.shape == (B, S, D)

    bqkv = np.asarray(bqkv, dtype=np.float32)
    bq = np.asarray(bq, dtype=np.float32)
    bproj = np.asarray(bproj, dtype=np.float32)
    in_maps = shard_inputs(x, Wqkv, Wq, Wproj, bqkv, bq)
    if _cached_nc is None:
        _cached_nc = build_kernel()

    def run_once():
        res = run_bass_kernel_spmd(_cached_nc, in_maps,
                                   core_ids=list(range(NCORES)))
        o = np.empty((B, S, D), dtype=np.float32)
        for c in range(NCORES):
            b, blk = c // 4, c % 4
            o[b, blk * NQ:(blk + 1) * NQ, :] = res.results[c]["out"]
        # K bias cancels in softmax (constant per-query score shift).
        # V bias adds bv to every attention output -> exact bv@Wproj add.
        o += bqkv[2 * D:3 * D] @ Wproj
        o += bproj
        return o

    # one-row host probe guards against rare transient device corruption
    def probe_err(o):
        r = A  # first non-anchor row of batch 0
        anch = x[0, :A, :]
        Km = anch @ Wqkv[:, D:2 * D] + bqkv[D:2 * D]
        Vm = anch @ Wqkv[:, 2 * D:3 * D] + bqkv[2 * D:3 * D]
        q = x[0, r, :] @ Wq + bq
        row = np.empty(D, np.float32)
        for h in range(H):
            sl = slice(h * HD, (h + 1) * HD)
            s = (Km[:, sl] @ q[sl]) * SCALE
            e = np.exp(s - s.max())
            row[sl] = (e / e.sum()) @ Vm[:, sl]
        ref_row = row @ Wproj + bproj
        return (np.linalg.norm(o[0, r] - ref_row)
                / max(np.linalg.norm(ref_row), 1e-6))

    out = run_once()
    if not np.isfinite(out).all() or probe_err(out) > 5e-2:
        out = run_once()
    return out

